# revision 28
# baseline (speedup 1.0000x reference)
"""BitLinear 1.58 (nn_BitLinear158) Trainium2 Bass kernel.

Problem: x:[4,2048,4096] f32, weight:[4096,4096] f32 ->
         absmax-group-quantized x (8-bit fake quant, groups of 64) @
         ternary-quantized weight.T (per-row absmean scale) -> [4,2048,4096].

Sharding: data-parallel over tokens (1024 tokens/core, full weight
replicated) — minimizes replicated elementwise work.

Per-core kernel (M=1024, K=4096, O=4096), engine-balanced so the tensor
engine (437us roofline at 2.4GHz) paces the pipeline:
  - PE: stationary = ternary weights [k,128o], moving = x_q [k,512m],
    psum [o,m]; 2048 matmuls stream at ~216ns each (full clock,
    ldweights pipelined).
  - DVE: the group reduces (free-axis reduces are DVE-only), the scale
    chains, the x scale/round passes, and the sign-sum + grid-of-2
    boundary fix (fp16 2x/4x DVE modes).
  - Act (scalar): one Sign pass per threshold per w row-block
    (ternarize via sign(w-s/2)+sign(w+s/2)) and psum eviction via Copy
    activation with scale=0.5*s as a per-partition AP (the [o,m] psum
    orientation makes the row scale per-partition).
  - SP (sync): all loads and all xbar transposes (keeping transpose
    issue off the Act queue, which would head-of-line block behind
    evictions); gpsimd: output stores via SWDGE.
  - Full [128,4096] row tiles everywhere: one DMA load, one transpose,
    one add/fix per block — minimizes instruction count and shared
    HWDGE-ring packet pressure (all hw DMAs share one ring).
  - The weight pipeline is software-pipelined: stage A (reduce + scale
    chain + Sign issues) runs two ocs ahead of the matmuls, stage B
    (sign-sum + fix + transpose) one oc ahead, evictions one oc behind
    — no engine queue head-of-line blocks. Weight staging rotates
    across two pools (4-oc depth).
  - SBUF is the binding constraint: the x staging pools live in a head
    scope released after activation quant; the sg/ev pools live in the
    body scope.
  - s computed with a two-stage compensated reduction (exact 2^-12-grid
    split) to track the f32 reference mean closely; ternary decisions
    are exact comparisons against +-0.5*s so there is no divide
    rounding. Both are load-bearing: seed-0 has weights within one f32
    ulp of the |w|=0.5s boundary.
  - gpsimd/Pool compute measured and rejected: Pool TENSOR_SCALAR runs
    ~29us/[128,2048] (software DSP path) and even Pool TENSOR_TENSOR
    (~3.6us) degrades concurrent DVE throughput ~2x via SBUF port
    contention.
"""
import sys

sys.path.insert(0, "/opt/trn_rl_repo")

import numpy as np

B, S, D_IN, D_OUT = 4, 2048, 4096, 4096
N_CORES = 8
M_TOT = B * S
M_C = M_TOT // N_CORES

P = 128
G = 64
MAGIC = float(1.5 * 2.0 ** 23)   # fp32 round-to-nearest-even trick
MAGIC2 = float(1.5 * 2.0 ** 11)  # quantize-to-2^-12-grid trick
FIXC = float(1.5 * 2.0 ** 24)    # f32 grid-of-2 round (ties-to-even)
EPS = 1e-5
QMAX = 127.0
INV_QMAX = float(np.float32(1.0 / 127.0))

_cache = {}


def _build(M, K, O):
    import concourse.bass as bass
    import concourse.tile as tile
    from concourse import bacc, mybir

    f32 = mybir.dt.float32
    f16 = mybir.dt.float16
    Alu = mybir.AluOpType
    Act = mybir.ActivationFunctionType
    Ax = mybir.AxisListType

    KSUB = K // P        # 32 contraction chunks
    MB = M // P          # token blocks
    OC = O // P          # out-feature blocks
    MH = M // 512        # psum column halves
    NG = K // G          # 64 quant/absmean groups per row

    nc = bacc.Bacc("TRN2", target_bir_lowering=False, num_devices=1)
    x = nc.dram_tensor("x", [M, K], f32, kind="ExternalInput")
    w = nc.dram_tensor("w", [O, K], f32, kind="ExternalInput")
    # transposed output [O, M]; host untransposes at gather time
    out = nc.dram_tensor("out", [O, M], f32, kind="ExternalOutput")

    xap, wap, oap = x.ap(), w.ap(), out.ap()

    with tile.TileContext(nc) as tc:
        with (
            tc.tile_pool(name="xq", bufs=1) as xq_pool,
            tc.tile_pool(name="wstA", bufs=2) as wstA,
            tc.tile_pool(name="wt", bufs=4) as wt_pool,
            tc.tile_pool(name="small", bufs=3) as small,
            tc.tile_pool(name="sv", bufs=8) as sv_pool,
            tc.tile_pool(name="ps", bufs=8, space="PSUM") as ps_pool,
        ):
            xq_t = xq_pool.tile([P, KSUB, M], f16, name="xq_t")
            w_stage = {}
            wst_pools = [wstA, None]  # [1] set to wstB inside the body scope

            def wload(oc):
                # consecutive oc pairs alternate pools: 4-oc rotation depth
                pool = wst_pools[(oc // 2) % 2] or wstA
                wh = pool.tile([P, K], f32, tag="wst", name=f"wh{oc}")
                nc.sync.dma_start(wh[:], wap[oc * P:(oc + 1) * P, :])
                w_stage[oc] = wh

            # ---- head scope: x staging pools, released after quant ----
            with (
                tc.tile_pool(name="xst", bufs=3) as xst,
                tc.tile_pool(name="xq16", bufs=2) as xq16_pool,
            ):
                x_stage = {}
                for mb in range(MB):
                    xt = xst.tile([P, K], f32, tag="xst", name=f"xt{mb}")
                    nc.sync.dma_start(xt[:], xap[mb * P:(mb + 1) * P, :])
                    x_stage[mb] = xt

                for _oc in range(min(2, OC)):
                    wload(_oc)

                for mb in range(MB):
                    xt = x_stage.pop(mb)
                    xg = xt.rearrange("p (g e) -> p g e", e=G)
                    am = small.tile([P, NG], f32, tag="am")
                    nc.vector.tensor_reduce(am[:], xg, Ax.X, Alu.max,
                                            apply_absolute_value=True)
                    am2 = small.tile([P, NG], f32, tag="am2")
                    nc.vector.tensor_scalar(am2[:], am[:], EPS, None,
                                            Alu.max)
                    rc = small.tile([P, NG], f32, tag="rc")
                    nc.vector.reciprocal(rc[:], am2[:])
                    scale = small.tile([P, NG], f32, tag="scale")
                    nc.vector.tensor_scalar(scale[:], rc[:], QMAX, None,
                                            Alu.mult)
                    inv = small.tile([P, NG], f32, tag="inv")
                    nc.vector.tensor_scalar(inv[:], am2[:], INV_QMAX,
                                            None, Alu.mult)
                    # xs = x * scale (group-broadcast)
                    nc.vector.tensor_tensor(
                        xg, xg,
                        scale[:, :, None].to_broadcast((P, NG, G)),
                        Alu.mult)
                    # q = rint(xs) via magic add/sub
                    nc.vector.tensor_scalar(xt[:], xt[:], MAGIC, MAGIC,
                                            Alu.add, Alu.subtract)
                    # x_q = q * (absmax/127) -> fp16
                    xq16 = xq16_pool.tile([P, K], f16, tag="xq16",
                                          name=f"xq16_{mb}")
                    nc.vector.tensor_tensor(
                        xq16.rearrange("p (g e) -> p g e", e=G), xg,
                        inv[:, :, None].to_broadcast((P, NG, G)),
                        Alu.mult)
                    nc.sync.dma_start_transpose(
                        xq_t[:, :, mb * P:(mb + 1) * P], xq16[:])

            # ---- body scope: second weight pool + sign/evict pools ----
            with (
                tc.tile_pool(name="wstB", bufs=2) as wstB,
                tc.tile_pool(name="sg", bufs=2) as sg_pool,
                tc.tile_pool(name="ev", bufs=2) as ev_pool,
            ):
                wst_pools[1] = wstB

                wt_tiles = {}
                bp_tiles = {}
                sg_tiles = {}

                def wternA(oc):
                    """Reduce + compensated scale chain + Sign issues."""
                    wh = w_stage.pop(oc)
                    gs = small.tile([P, NG], f32, tag="gs")
                    nc.vector.tensor_reduce(
                        gs[:], wh.rearrange("p (g e) -> p g e", e=G),
                        Ax.X, Alu.add, apply_absolute_value=True)
                    # s = max(mean|row|, eps), two-stage compensated sum
                    hql = small.tile([P, 2 * NG], f32, tag="hql")
                    nc.vector.tensor_scalar(hql[:, :NG], gs[:], MAGIC2,
                                            MAGIC2, Alu.add, Alu.subtract)
                    nc.vector.tensor_tensor(hql[:, NG:], gs[:],
                                            hql[:, :NG], Alu.subtract)
                    shl = small.tile([P, 2], f32, tag="shl")
                    nc.vector.tensor_reduce(
                        shl[:], hql.rearrange("p (a b) -> p a b", a=2),
                        Ax.X, Alu.add)
                    ssum = small.tile([P, 1], f32, tag="ssum")
                    nc.vector.tensor_tensor(ssum[:], shl[:, 0:1],
                                            shl[:, 1:2], Alu.add)
                    sv = small.tile([P, 1], f32, tag="svv")
                    nc.vector.tensor_scalar(sv[:], ssum[:],
                                            float(np.float32(1.0 / K)),
                                            EPS, Alu.mult, Alu.max)
                    # eviction scale is 0.5*s (the sign-sum is 2t)
                    bp = sv_pool.tile([P, 1], f32, tag="bp",
                                      name=f"bp{oc}")
                    nc.vector.tensor_scalar(bp[:], sv[:], 0.5, None,
                                            Alu.mult)
                    bp_tiles[oc] = bp
                    bn = small.tile([P, 1], f32, tag="bn")
                    nc.vector.tensor_scalar(bn[:], sv[:], -0.5, None,
                                            Alu.mult)
                    # 2t = sign(w-0.5s) + sign(w+0.5s); exact comparisons.
                    sga = sg_pool.tile([P, K], f16, tag="sga",
                                       name=f"sga{oc}")
                    nc.scalar.activation(out=sga[:], in_=wh[:],
                                         func=Act.Sign, bias=bn[:],
                                         scale=1.0)
                    sgb = sg_pool.tile([P, K], f16, tag="sgb",
                                       name=f"sgb{oc}")
                    nc.scalar.activation(out=sgb[:], in_=wh[:],
                                         func=Act.Sign, bias=bp[:],
                                         scale=1.0)
                    sg_tiles[oc] = (sga, sgb)

                def wternB(oc):
                    """Sign-sum + boundary fix (maps +-1 -> 0) + transpose."""
                    sga, sgb = sg_tiles.pop(oc)
                    wt = wt_pool.tile([P, KSUB, P], f16, tag="wt",
                                      name=f"wt{oc}")
                    wt_tiles[oc] = wt
                    nc.vector.tensor_tensor(sga[:], sga[:], sgb[:],
                                            Alu.add)
                    nc.vector.tensor_scalar(sga[:], sga[:], FIXC, FIXC,
                                            Alu.add, Alu.subtract)
                    nc.sync.dma_start_transpose(wt[:, :, :], sga[:])

                ps_tiles = {}

                def evict(oc):
                    bp = bp_tiles.pop(oc)
                    for mh in range(MH):
                        ps = ps_tiles.pop((oc, mh))
                        ev = ev_pool.tile([P, 512], f32)
                        nc.scalar.activation(out=ev[:], in_=ps[:],
                                             func=Act.Copy, scale=bp[:])
                        nc.gpsimd.dma_start(
                            oap[oc * P:(oc + 1) * P,
                                mh * 512:(mh + 1) * 512], ev[:])

                for _oc in range(2, min(4, OC)):
                    wload(_oc)
                wternA(0)
                wternB(0)
                if OC > 1:
                    wternA(1)
                for oc in range(OC):
                    if oc + 4 < OC:
                        wload(oc + 4)
                    if oc + 2 < OC:
                        wternA(oc + 2)
                    if oc + 1 < OC:
                        wternB(oc + 1)
                    wt = wt_tiles.pop(oc)
                    for mh in range(MH):
                        ps = ps_pool.tile([P, 512], f32)
                        ps_tiles[(oc, mh)] = ps
                        for ks in range(KSUB):
                            nc.tensor.matmul(
                                ps[:], wt[:, ks, :],
                                xq_t[:, ks, mh * 512:(mh + 1) * 512],
                                start=(ks == 0), stop=(ks == KSUB - 1))
                    if oc >= 1:
                        evict(oc - 1)
                evict(OC - 1)

    nc.compile()
    return nc


def _get_nc():
    if "nc" not in _cache:
        _cache["nc"] = _build(M_C, D_IN, D_OUT)
    return _cache["nc"]


def run(x, weight, trace=False):
    """Run on 8 NeuronCores; returns (full output [B,S,D_OUT], results obj)."""
    from concourse.bass_utils import run_bass_kernel_spmd

    x = np.ascontiguousarray(np.asarray(x, dtype=np.float32))
    w = np.ascontiguousarray(np.asarray(weight, dtype=np.float32))
    assert x.shape == (B, S, D_IN) and w.shape == (D_OUT, D_IN)
    xf = x.reshape(M_TOT, D_IN)
    nc = _get_nc()
    in_maps = [
        {"x": np.ascontiguousarray(xf[c * M_C:(c + 1) * M_C]), "w": w}
        for c in range(N_CORES)
    ]
    res = run_bass_kernel_spmd(nc, in_maps, core_ids=list(range(N_CORES)),
                               trace=trace)
    outf = np.concatenate(
        [res.results[c]["out"].T for c in range(N_CORES)], axis=0)
    return np.ascontiguousarray(outf).reshape(B, S, D_OUT), res


def kernel(x, weight):
    out, _ = run(x, weight)
    return out
